# revision 37
# baseline (speedup 1.0000x reference)
"""EnergyHead kernel for Trainium2 (8 NeuronCores, batch-parallel), fp8 edition.

Computes, per batch element:
    xH = x @ W_H.T
    scores = x @ xH.T  (strict lower-triangular causal mask, diag excluded)
    wei = softmax(scores); fully-masked row 0 zeroed
    out = -(wei @ xH)

All three matmul phases run in fp8e4 (e4m3) with DoubleRow perf mode
(K=256 per instruction at 0.5 cycles/row = 4x the f32r/bf16 PE throughput),
using error-compensated operand splitting: each tensor is a power-of-2
scaled (value, residual) e4m3 pair, and each product keeps the three
dominant cross terms (v*v + v*r + r*v), dropping only the r*r term.
Power-of-2 scales (x*16, W*512, xH*16) put every plane in e4m3's normal
range and make all kept terms share one PSUM accumulation scale.
Phase 3 uses 2 terms (wei single-plane); its softmax normalizer is the sum
of the *quantized* weights (computed on the PE against a ones vector), so
the dominant-weight quantization error cancels. End-to-end rel err ~1e-2.

Sharding: data-parallel over B=8 across 8 cores. Host pre-quantizes x / W
into byte-interleaved (value, residual) fp8 pair layouts; xH pairs are
produced on-device the same way so one uint16-viewed DMA transpose moves
both planes into the natural layout for phase 3.

Schedule notes: PE p-state resets on any idle gap (post-gap matmuls run
~2-3.7x slower while re-ramping), so the emission order keeps PE
continuously busy: warm-up matmuls cover the input-DMA window (two wide
gate DMAs: W cols then x's first t-block), q-tiles run mid-size-first,
ramp to the largest, and finish on the smallest (shortest drain chain).
Per-block reduce_max runs at scores time; exp / transpose / fp8-quantize
interleave so each tile's softmax chain hides under the next tiles'
scores matmuls. Tiles with <=CFG_PET s-chunks transpose their weights on
the PE (fp8 is_transpose vs an identity, psum copied out on DVE) instead
of DMA+Pool, cutting the chain latency that dominates the small-tile
tail; the last CFG_DVEOSB tiles split their two out-scale halves across
ACT and DVE so the final stores drain in parallel.
"""
import sys
import os
import functools

sys.path.insert(0, "/opt/trn_rl_repo")
import numpy as np
import ml_dtypes

E4NP = ml_dtypes.float8_e4m3

CFG_DEPTH = int(os.environ.get("K_DEPTH", "3"))      # initial pending tiles
CFG_MINPB = int(os.environ.get("K_MINPB", "8"))      # top-up pending-block floor
CFG_WARM = int(os.environ.get("K_WARM", "12"))
CFG_ORDER = os.environ.get("K_ORDER", "6,7,8,9,11,12,10,14,15,13,5,4,3,1,2,0")

B, T, C = 8, 2048, 1024
NCORES = 8
P = 128
QT = T // P                  # 16 q-tiles
CP = C // 256                # 4 c-pair chunks (K=256 each)
NEG_BIG = -1e30
XS, WS, HS = 16.0, 512.0, 16.0
# P1 psum scale = XS*WS = 8192 -> xH16 copy scale = HS/8192 = 1/512
# P2 psum scale = XS*HS = 256  -> exp scale = 1/256
# P3 psum scale = HS    = 16   -> folded into recip = -1/(16*sums)


def _sblocks(i):
    S = P * (i + 1)
    k, rem = divmod(S, 512)
    return [512] * k + ([rem] if rem else [])


def _pieces(n):
    """Split a block of n output cols into <=256-wide DoubleRow pieces."""
    out, off = [], 0
    while n > 0:
        m = min(256, n)
        out.append((off, m))
        off += m
        n -= m
    return out


def _order():
    if CFG_ORDER == "desc":
        return list(range(QT - 1, 0, -1)) + [0]
    if CFG_ORDER == "asc":
        return list(range(1, QT)) + [0]
    return [int(v) for v in CFG_ORDER.split(",")]


@functools.lru_cache(maxsize=4)
def _build():
    import concourse.bacc as bacc
    import concourse.tile as tile
    from concourse import mybir

    f32 = mybir.dt.float32
    f8 = mybir.dt.float8e4
    bf16 = mybir.dt.bfloat16
    fp16 = mybir.dt.float16
    u16 = mybir.dt.uint16
    X = mybir.AxisListType.X
    Exp = mybir.ActivationFunctionType.Exp
    Copy = mybir.ActivationFunctionType.Copy
    DR = mybir.MatmulPerfMode.DoubleRow
    Alu = mybir.AluOpType

    nc = bacc.Bacc("TRN2", target_bir_lowering=False, debug=False,
                   enable_asserts=False, num_devices=NCORES)

    # byte-interleaved (value, residual) pair layouts; row r=128p+k holds
    # c-chunk pair p, free col j*(2F) + 2f + byte
    x8_d = nc.dram_tensor("x8", [P, CP * 4 * T], f8, kind="ExternalInput").ap()
    w8_d = nc.dram_tensor("w8", [P, CP * 4 * C], f8, kind="ExternalInput").ap()
    out_d = nc.dram_tensor("out", [T, C], fp16, kind="ExternalOutput").ap()

    with tile.TileContext(nc) as tc:
        with tc.tile_pool(name="pers", bufs=1) as pers, \
             tc.tile_pool(name="stats", bufs=8) as statsp, \
             tc.tile_pool(name="blk", bufs=8, space="PSUM") as ps512, \
             tc.tile_pool(name="wei", bufs=2) as weip, \
             tc.tile_pool(name="weiT", bufs=2) as weitp, \
             tc.tile_pool(name="weiT8", bufs=2) as weit8p, \
             tc.tile_pool(name="osb", bufs=4) as outsp:

            # warm-up scratch on DVE so PE can start immediately
            warm = pers.tile([P, 512], bf16, tag="warm")
            nc.vector.memset(warm[:], 1.0)
            if CFG_WARM:
                wps = ps512.tile([P, 512], f32, tag="blk")
                for _k in range(CFG_WARM):
                    nc.tensor.matmul(wps[:], warm[:, 0:P], warm[:],
                                     start=True, stop=True)

            # ---- input DMAs, all on the sync(SP) HWDGE ring, in need-order:
            # w8 first halves + x8 tb0 gate the first real matmul
            x8_all = pers.tile([P, CP, 2, T, 2], f8, tag="x8", name="x8")
            w8_all = pers.tile([P, CP, 2, C, 2], f8, tag="w8", name="w8")
            x8_sb = [x8_all[:, p] for p in range(CP)]
            w8_sb = [w8_all[:, p] for p in range(CP)]
            x8d_v = x8_d.rearrange("k (p j t2) -> k p j t2", p=CP, j=2)
            w8d_v = w8_d.rearrange("k (p j c2) -> k p j c2", p=CP, j=2)
            # minimal first-matmul gate: w8 d[0:256] + x8 tb0, one wide DMA
            # each; the rest paced behind them in need-order
            nc.sync.dma_start(w8_all[:, :, :, 0:256, :], w8d_v[:, :, :, 0:512])
            nc.sync.dma_start(x8_all[:, :, :, 0:512, :], x8d_v[:, :, :, 0:1024])
            nc.sync.dma_start(w8_all[:, :, :, 256:512, :],
                              w8d_v[:, :, :, 512:1024])
            nc.sync.dma_start(w8_all[:, :, :, 512:1024, :],
                              w8d_v[:, :, :, 1024:2048])
            for tb in range(1, 4):
                nc.sync.dma_start(
                    x8_all[:, :, :, 512 * tb:512 * (tb + 1), :],
                    x8d_v[:, :, :, 1024 * tb:1024 * (tb + 1)])

            # ---- constants (Pool, after the warm memset)
            diagmask = pers.tile([P, P], f32, tag="diagmask")
            nc.gpsimd.memset(diagmask[:], 0.0)
            nc.gpsimd.affine_select(
                out=diagmask[:], in_=diagmask[:],
                compare_op=mybir.AluOpType.is_gt,
                fill=NEG_BIG, base=0, pattern=[[-1, P]], channel_multiplier=1,
            )
            # +1 everywhere except row 0 (zeroes the fully-masked first row)
            rowmask0 = pers.tile([P, 1], f32, tag="rowmask0")
            nc.gpsimd.memset(rowmask0[:], 1.0)
            nc.gpsimd.memset(rowmask0[0:1, :], 0.0)
            ones8 = pers.tile([P, 2, 1], f8, tag="ones8")
            nc.gpsimd.memset(ones8[:], 1.0)

            stag8 = [pers.tile([P, 2, T, 2], f8, tag=f"st8_{p}", name=f"st8_{p}")
                     for p in range(CP)]
            # natural-layout xH pairs [t', t-chunk, c, byte]; chunk QT is a
            # zero pad so odd-length phase-3 contractions stay DoubleRow
            xHv8 = pers.tile([P, QT + 1, C, 2], f8, tag="xHv8")
            nc.gpsimd.memset(xHv8[:, QT, :, :], 0.0)

            # ---- phase 1: xHT pairs, interleaved into stag8.
            # psum(tb,d) = sum over c-pairs of 3-term DoubleRow products
            TERMS = ((0, 0), (0, 1), (1, 0))  # (w byte, x byte)
            for tb in range(4):
                for d in range(8):
                    pmm = ps512.tile([P, 512], f32, tag="blk")
                    nmm = 2 * len(TERMS) * CP
                    k = 0
                    for h in range(2):
                        toff = 512 * tb + 256 * h
                        for bw, bx in TERMS:
                            for p in range(CP):
                                nc.tensor.matmul(
                                    pmm[:, 256 * h:256 * (h + 1)],
                                    w8_sb[p][:, :, P * d:P * (d + 1), bw],
                                    x8_sb[p][:, :, toff:toff + 256, bx],
                                    start=(k == 0), stop=(k == nmm - 1),
                                    perf_mode=DR, skip_group_check=True)
                                k += 1
                    dst = stag8[d // 2][:, d % 2, 512 * tb:512 * (tb + 1), :]
                    nc.scalar.activation(dst[:, :, 0], pmm[:], Copy,
                                         bias=0.0, scale=1.0 / 512.0)
                    nc.vector.scalar_tensor_tensor(
                        out=dst[:, :, 1], in0=pmm[:], scalar=1.0 / 512.0,
                        op0=Alu.mult, op1=Alu.subtract, in1=dst[:, :, 0])
                # after tb1/tb3: transpose finished 1024-col groups into
                # natural layout (both planes at once via the u16 view)
                if tb % 2 == 1:
                    g = tb // 2
                    for p in range(CP):
                        for j in range(2):
                            d = 2 * p + j
                            src = stag8[p][:, j, 1024 * g:1024 * (g + 1), :]
                            nc.scalar.dma_start_transpose(
                                xHv8[:, 8 * g:8 * (g + 1), P * d:P * (d + 1), :]
                                .rearrange("k t c b -> k t (c b)").bitcast(u16),
                                src.rearrange("k t b -> k (t b)").bitcast(u16))

            # ---- q-tile loop (software-pipelined emission)
            def emit_scores(i):
                blks = []
                negblk = statsp.tile([P, 4], f32, tag="negblk")
                off = 0
                sb = _sblocks(i)
                for bi, n in enumerate(sb):
                    pmm = ps512.tile([P, 512], f32, tag="blk")
                    pieces = _pieces(n)
                    nmm = 3 * CP * len(pieces)
                    k = 0
                    for poff, m in pieces:
                        for bx, bh in TERMS:
                            for p in range(CP):
                                nc.tensor.matmul(
                                    pmm[:, poff:poff + m],
                                    x8_sb[p][:, :, P * i:P * (i + 1), bx],
                                    stag8[p][:, :, off + poff:off + poff + m, bh],
                                    start=(k == 0), stop=(k == nmm - 1),
                                    perf_mode=DR, skip_group_check=True)
                                k += 1
                    if bi == len(sb) - 1:
                        # mask the diagonal block (last 128 columns)
                        nc.vector.tensor_add(pmm[:, n - P:n], pmm[:, n - P:n],
                                             diagmask[:])
                    # per-block max at scores time so the softmax chain later
                    # starts at the cross-block reduce
                    nc.vector.reduce_max(negblk[:, bi:bi + 1], pmm[:, :n],
                                         axis=X, negate=True)
                    blks.append((pmm, off, n))
                    off += n
                return negblk, blks

            def emit_wei(i, negblk, blks):
                """Softmax chain through the quantized transposed weights."""
                nblk = len(blks)
                nms = statsp.tile([P, 1], f32, tag="nms")
                if nblk > 1:
                    gneg = statsp.tile([P, 1], f32, tag="gneg")
                    nc.vector.tensor_reduce(gneg[:], negblk[:, :nblk],
                                            axis=X, op=Alu.min)
                    nc.vector.tensor_scalar_mul(nms[:], gneg[:], 1.0 / 256.0)
                else:
                    nc.vector.tensor_scalar_mul(nms[:], negblk[:, 0:1],
                                                1.0 / 256.0)

                S = P * (i + 1)
                nk = i + 1
                pad = nk % 2
                nkp = nk + pad
                weiT8 = weit8p.tile([P, nkp, P], f8, tag="weiT8")

                if nk <= CFG_PET:
                    # small tile: exp straight to fp8, transpose on the PE
                    # (is_transpose matmul vs identity), copy psum->sbuf on
                    # DVE. Avoids the DMA-transpose + Pool-quantize latency.
                    wei8 = weip.tile([P, S], f8, tag="wei8")
                    for k, (pmm, off, n) in enumerate(blks):
                        nc.scalar.activation(wei8[:, off:off + n], pmm[:, :n],
                                             Exp, bias=nms[:],
                                             scale=1.0 / 256.0)
                    for k in range(nk):
                        psT = ps512.tile([P, 512], f32, tag="blk")
                        # fp8 transpose mode requires output element step 2
                        psT8 = psT[:].bitcast(f8).rearrange(
                            "p (n b) -> p n b", b=2)
                        nc.tensor.matmul(psT8[:, 0:P, 0],
                                         wei8[:, P * k:P * (k + 1)],
                                         ident8[:], is_transpose=True)
                        nc.vector.tensor_copy(weiT8[:, k, :],
                                              psT8[:, 0:P, 0])
                    if pad:
                        nc.gpsimd.memset(weiT8[:, nk, :], 0.0)
                    return weiT8

                wei = weip.tile([P, S], bf16, tag="wei")
                weiT3 = weitp.tile([P, nk, P], bf16, tag="weiT")
                h1 = (nk // 2) * P

                # exp blocks, with the transpose+quantize halves interleaved
                # as soon as their source columns are ready
                t1_done = False
                for k, (pmm, off, n) in enumerate(blks):
                    nc.scalar.activation(wei[:, off:off + n], pmm[:, :n], Exp,
                                         bias=nms[:], scale=1.0 / 256.0)
                    if not t1_done and nk >= 2 and off + n >= h1:
                        nc.scalar.dma_start_transpose(
                            weiT3[:, :nk // 2, :], wei[:, :h1])
                        for q0 in range(0, nk // 2, 2):
                            q1 = min(q0 + 2, nk // 2)
                            nc.gpsimd.tensor_copy(weiT8[:, q0:q1, :],
                                                  weiT3[:, q0:q1, :])
                        t1_done = True
                if nk >= 2:
                    nc.scalar.dma_start_transpose(
                        weiT3[:, nk // 2:, :], wei[:, h1:S])
                    for q0 in range(nk // 2, nk, 2):
                        q1 = min(q0 + 2, nk)
                        nc.gpsimd.tensor_copy(weiT8[:, q0:q1, :],
                                              weiT3[:, q0:q1, :])
                else:
                    nc.scalar.dma_start_transpose(weiT3[:], wei[:, :S])
                    nc.gpsimd.tensor_copy(weiT8[:, :nk, :], weiT3[:])
                if pad:
                    nc.gpsimd.memset(weiT8[:, nk, :], 0.0)
                return weiT8

            def emit_out(i, weiT8, dve_osb=False, all_dve=False):
                nk = i + 1
                nkp = nk + nk % 2
                # quantized-weight row sums on the PE (vs ones)
                psum_s = ps512.tile([P, 512], f32, tag="blk")
                for k2 in range(nkp // 2):
                    nc.tensor.matmul(
                        psum_s[:, 0:1], weiT8[:, 2 * k2:2 * k2 + 2, :],
                        ones8[:], start=(k2 == 0), stop=(k2 == nkp // 2 - 1),
                        perf_mode=DR, skip_group_check=True)
                srecip = statsp.tile([P, 1], f32, tag="srecip")
                nc.vector.tensor_scalar_mul(srecip[:], psum_s[:, 0:1], -16.0)
                recip = statsp.tile([P, 1], f32, tag="recip")
                nc.vector.reciprocal(recip[:], srecip[:])
                if i == 0:
                    nc.vector.tensor_mul(recip[:], recip[:], rowmask0[:])

                # out = wei8 @ (xHq + xHr), 2-term DoubleRow over s-pairs
                osb = outsp.tile([P, C], fp16, tag="osb")
                for cb in range(2):
                    opc = ps512.tile([P, 512], f32, tag="blk")
                    nmm = 2 * 2 * (nkp // 2)
                    k = 0
                    for h in range(2):
                        coff = 512 * cb + 256 * h
                        for bh in range(2):
                            for k2 in range(nkp // 2):
                                nc.tensor.matmul(
                                    opc[:, 256 * h:256 * (h + 1)],
                                    weiT8[:, 2 * k2:2 * k2 + 2, :],
                                    xHv8[:, 2 * k2:2 * k2 + 2,
                                         coff:coff + 256, bh],
                                    start=(k == 0), stop=(k == nmm - 1),
                                    perf_mode=DR, skip_group_check=True)
                                k += 1
                    sl = osb[:, 512 * cb:512 * (cb + 1)]
                    if all_dve or (cb == 1 and dve_osb):
                        # scale halves on DVE so the final tiles' scale
                        # copies drain in parallel with ACT's
                        nc.vector.tensor_scalar_mul(sl, opc[:], recip[:])
                    else:
                        nc.scalar.activation(sl, opc[:], Copy, bias=0.0,
                                             scale=recip[:])
                nc.sync.dma_start(out_d[P * i:P * (i + 1), :], osb[:])

            # adaptive software pipeline: keep CFG_DEPTH tiles of scores
            # pending ahead of the softmax/out emission, plus top up while
            # the pending psum-block count stays below CFG_MINPB (small
            # tiles prefetch deeper; big tiles are bounded by PSUM banks)
            order = _order()
            pending = []
            pb = 0
            nexti = 0

            def top_up(force):
                nonlocal pb, nexti
                while nexti < QT and (
                        len(pending) < force
                        or pb + len(_sblocks(order[nexti])) <= CFG_MINPB):
                    t = order[nexti]
                    pending.append((t,) + emit_scores(t))
                    pb += len(_sblocks(t))
                    nexti += 1

            top_up(CFG_DEPTH)
            ndone = 0
            while pending:
                t, negblk, blks = pending.pop(0)
                pb -= len(blks)
                top_up(CFG_DEPTH + 1 if pending else CFG_DEPTH)
                w8t = emit_wei(t, negblk, blks)
                ndone += 1
                emit_out(t, w8t, dve_osb=(QT - ndone < CFG_DVEOSB),
                         all_dve=(ndone == QT and CFG_LASTDVE))

    nc.compile()
    return nc


def _fp8_pair_interleave(a: np.ndarray, scale: float) -> np.ndarray:
    """[C, F] f32 -> [128, C//64*F] e4m3: row k, col (p, j, f, byte) —
    c-chunk pair p, sub-chunk j, byte0=value, byte1=residual. Matches the
    on-chip single-tile layout so any slice DMA balances in <=3 dims."""
    Cd, F = a.shape
    s = np.asarray(a, dtype=np.float32) * scale
    q = s.astype(E4NP)
    r = (s - q.astype(np.float32)).astype(E4NP)
    pair = np.stack([q, r], axis=-1)              # [C, F, 2]
    pair = pair.reshape(Cd // 256, 2, 128, F, 2)  # [p, j, k, f, byte]
    pair = pair.transpose(2, 0, 1, 3, 4)          # [k, p, j, f, byte]
    return np.ascontiguousarray(pair).reshape(128, (Cd // 64) * F)


def _prep_inputs(x: np.ndarray, W_H: np.ndarray) -> list[dict]:
    x = np.asarray(x, dtype=np.float32)
    W_H = np.asarray(W_H, dtype=np.float32)
    w8 = _fp8_pair_interleave(W_H.T, WS)          # wT layout [c, d]
    return [{"x8": _fp8_pair_interleave(x[b].T, XS), "w8": w8}
            for b in range(B)]


def kernel(x: np.ndarray, W_H: np.ndarray) -> np.ndarray:
    from concourse import bass_utils

    nc = _build()
    in_maps = _prep_inputs(x, W_H)
    res = bass_utils.run_bass_kernel_spmd(nc, in_maps,
                                          core_ids=list(range(NCORES)))
    return np.stack([res.results[b]["out"].astype(np.float32)
                     for b in range(B)])


if __name__ == "__main__":
    x = np.random.randn(B, T, C).astype(np.float32)
    W = (np.random.randn(C, C) / np.sqrt(C)).astype(np.float32)
    out = kernel(x, W)
    print("out", out.shape, out.dtype)
